# revision 23
# baseline (speedup 1.0000x reference)
"""Trainium2 Bass kernel: hetero GraphConv (6 relations) + ReLU + 2-layer LSTM.

Strategy: shard destination nodes across 8 NeuronCores. Each core holds the
full source feature tables in HBM, gathers its incident edges' source rows
with indirect DMA, segment-sums them via one-hot scale-matrix matmuls into
PSUM, projects with the per-relation GraphConv weights (accumulated in PSUM,
ReLU+bias+mean fused on the scalar engine), then runs the 2-layer LSTM
data-parallel over its node rows.  No collectives are needed.

Data flows through the tensor engine in fp16 (features, scale matrices,
weights, LSTM states); all PSUM accumulation is fp32 and outputs are fp32.
"""
import sys

sys.path.insert(0, '/opt/trn_rl_repo')
import numpy as np

from concourse import bass, mybir, tile, bacc
from concourse.bass_utils import run_bass_kernel_spmd
from concourse.masks import make_identity

f32 = mybir.dt.float32
f16 = mybir.dt.float16
i32 = mybir.dt.int32
AF = mybir.ActivationFunctionType

P = 128

# relation -> (src type, dst type, weight column index)
RELS = {
    'in': ('pod', 'node', 0),
    'ni': ('node', 'pod', 1),
    'ii': ('pod', 'pod', 2),
    'si': ('svc', 'pod', 3),
    'sc': ('svc', 'svc', 4),
    'is': ('pod', 'svc', 5),
}
TYPE_RELS = {'node': ['in'], 'pod': ['ni', 'ii', 'si'], 'svc': ['sc', 'is']}
TYPE_IDX = {'node': 0, 'pod': 1, 'svc': 2}


class CFG:
    def __init__(self, n_node=500, n_pod=20000, n_svc=2000, T=16,
                 ncores=8, nodb=1, podb=20, svcb=2, lstm_grp=4):
        self.n = {'node': n_node, 'pod': n_pod, 'svc': n_svc}
        self.T = T
        self.F, self.H, self.O = 64, 128, 64
        self.TF, self.TH, self.TO = T * 64, T * 128, T * 64
        self.ncores = ncores
        self.nb = {'node': nodb, 'pod': podb, 'svc': svcb}
        # pods first and the gather-heavy node block last, so the first LSTM
        # pair's inputs are ready quickly and the node block's long gather
        # burst overlaps mid-kernel LSTM work
        self.blk0 = {'pod': 0, 'svc': podb, 'node': podb + svcb}
        self.nblk = nodb + podb + svcb
        self.nloc = self.nblk * P
        self.slot_cap = {ty: min(P, -(-self.n[ty] // (ncores * self.nb[ty])))
                         for ty in self.n}
        self.chunks = []
        b = 0
        while b < self.nblk:
            self.chunks.append(list(range(b, min(b + lstm_grp, self.nblk))))
            b += lstm_grp
        self.maxg = max(len(c) for c in self.chunks)

    def block_type(self, b):
        if b < self.blk0['svc']:
            return 'pod'
        if b < self.blk0['node']:
            return 'svc'
        return 'node'


def _pack_bins(degs, n_bins, slot_cap):
    """Greedy multi-constraint balancing of items (rows of degs) into bins."""
    n_items, R = degs.shape
    caps = np.maximum(degs.sum(0) / n_bins, 1.0)
    order = np.argsort(-(degs / caps).sum(1), kind='stable')
    bin_cnt = np.zeros((n_bins, R))
    bin_slots = np.zeros(n_bins, np.int64)
    assign = np.empty(n_items, np.int64)
    for i in order:
        load = ((bin_cnt + degs[i]) / caps).max(1) + bin_slots * 1e-5
        if (bin_slots >= slot_cap).any():
            load = np.where(bin_slots >= slot_cap, np.inf, load)
        b = int(np.argmin(load))
        assign[i] = b
        bin_cnt[b] += degs[i]
        bin_slots[b] += 1
    return assign


def preprocess(cfg, inputs):
    """Host-side graph partitioning. Returns per-core input maps + metadata."""
    edges = {}
    for r in RELS:
        src = np.asarray(inputs[f'{r}_src']).astype(np.int64)
        dst = np.asarray(inputs[f'{r}_dst']).astype(np.int64)
        edges[r] = (src, dst)

    # per-edge normalization scale (DGL norm='both')
    scale = {}
    for r, (sk, dk, _) in RELS.items():
        src, dst = edges[r]
        outd = np.maximum(np.bincount(src, minlength=cfg.n[sk]), 1).astype(np.float32)
        ind = np.maximum(np.bincount(dst, minlength=cfg.n[dk]), 1).astype(np.float32)
        rs_o = (1.0 / np.sqrt(outd)).astype(np.float32)
        rs_i = (1.0 / np.sqrt(ind)).astype(np.float32)
        scale[r] = (rs_o[src] * rs_i[dst]).astype(np.float32)

    # pack dst nodes of each type into (core, block) bins, balancing per-relation
    # in-degree so per-block chunk counts stay uniform
    assign, slot = {}, {}
    for ty in ('node', 'pod', 'svc'):
        rels = TYPE_RELS[ty]
        degs = np.stack(
            [np.bincount(edges[r][1], minlength=cfg.n[ty]) for r in rels], axis=1
        ).astype(np.float64)
        n_bins = cfg.ncores * cfg.nb[ty]
        a = _pack_bins(degs, n_bins, cfg.slot_cap[ty])
        assign[ty] = a
        s = np.zeros(cfg.n[ty], np.int64)
        for b in range(n_bins):
            ids = np.where(a == b)[0]
            s[ids] = np.arange(len(ids))
        slot[ty] = s

    # per (core, block, rel) edge lists; chunk counts maxed over cores
    by_cbr = {}
    for r, (sk, dk, _) in RELS.items():
        src, dst = edges[r]
        bins = assign[dk][dst]
        nbc = cfg.nb[dk]
        core = bins // nbc
        blk = cfg.blk0[dk] + bins % nbc
        sl = slot[dk][dst]
        order = np.lexsort((sl, blk, core))
        src, core, blk, sl, sc = (src[order], core[order], blk[order],
                                  sl[order], scale[r][order])
        for c in range(cfg.ncores):
            m = core == c
            by_cbr.setdefault(c, {})
            for b in range(cfg.nblk):
                mb = m & (blk == b)
                if not mb.any():
                    continue
                by_cbr[c].setdefault(b, {})[r] = (src[mb], sl[mb], sc[mb])

    K = []
    for b in range(cfg.nblk):
        bt = cfg.block_type(b)
        kb = {}
        for r in TYPE_RELS[bt]:
            mx = 1
            for c in range(cfg.ncores):
                e = by_cbr.get(c, {}).get(b, {}).get(r)
                if e is not None:
                    mx = max(mx, -(-len(e[0]) // P))
            kb[r] = mx
        K.append(kb)
    qtot = sum(sum(kb.values()) for kb in K)

    # flatten per-core gather indices + scale matrices in program order
    gidx = np.zeros((cfg.ncores, qtot, P), np.int32)
    s_all = np.zeros((cfg.ncores, qtot, P, P), np.float16)
    for c in range(cfg.ncores):
        q = 0
        for b in range(cfg.nblk):
            bt = cfg.block_type(b)
            for r in TYPE_RELS[bt]:
                e = by_cbr.get(c, {}).get(b, {}).get(r)
                if e is not None:
                    es, el, ev = e
                    n = len(es)
                    ch = q + np.arange(n) // P
                    ro = np.arange(n) % P
                    gidx[c, ch, ro] = es
                    s_all[c, ch, ro, el] = ev
                q += K[b][r]
        assert q == qtot

    # stacked projection weights: col block 0 = [W_ni;W_ii], 1 = [W_si;0],
    # 2 = [W_sc;W_is], 3 = [W_in;0] -- pairs contract as one K=128 matmul
    wg = np.zeros((128, 4 * cfg.H), np.float16)
    WI = {r: np.asarray(inputs[f'W_{r}'], np.float32) for r in RELS}
    wg[0:64, 0:128] = WI['ni']
    wg[64:128, 0:128] = WI['ii']
    wg[0:64, 128:256] = WI['si']
    wg[0:64, 256:384] = WI['sc']
    wg[64:128, 256:384] = WI['is']
    wg[0:64, 384:512] = WI['in']
    bg = np.zeros((P, 3), np.float32)
    for ty, rels in TYPE_RELS.items():
        bsum = sum(np.asarray(inputs[f'b_{r}'], np.float32) for r in rels)
        bg[:, TYPE_IDX[ty]] = bsum / len(rels)

    # LSTM gate rows reordered i,f,g,o -> f,i,o,g.  sigmoid(o) is computed as
    # tanh(o/2) alongside tanh(g) in one scale-vectored ACT op; the missing
    # (x+1)/2 affine is recovered by computing h' = 2h and folding 0.5 into
    # every consumer of h (Whh0, Wih1, Whh1 and the output ReLU scale).
    perm = np.r_[64:128, 0:64, 192:256, 128:192]
    wx0 = np.ascontiguousarray(
        np.asarray(inputs['Wih0'], np.float32).T[:, perm]).astype(np.float16)
    wr = np.ascontiguousarray(
        np.asarray(inputs['Whh0'], np.float32).T[:, perm] * 0.5
    ).astype(np.float16)  # [O, 4O]
    # layer1 x-input (h0) and recurrent (h1) weights stacked on K so both
    # contract in one K=128 matmul against the combined [h0;h1] state tile
    wl1 = np.ascontiguousarray(np.vstack(
        [np.asarray(inputs['Wih1'], np.float32).T[:, perm] * 0.5,
         np.asarray(inputs['Whh1'], np.float32).T[:, perm] * 0.5]
    )).astype(np.float16)  # [2O, 4O]
    b0 = (np.asarray(inputs['bih0'], np.float32)
          + np.asarray(inputs['bhh0'], np.float32))[perm]
    b1 = (np.asarray(inputs['bih1'], np.float32)
          + np.asarray(inputs['bhh1'], np.float32))[perm]
    b0[128:192] *= 0.5  # o-gate bias halved (tanh(x/2) trick)
    b1[128:192] *= 0.5
    tsc = np.concatenate([np.full(64, 0.5), np.ones(64)]).astype(np.float32)
    bl = np.stack([b0[:P], b0[P:], b1[:P], b1[P:], tsc], axis=1).astype(np.float32)

    xt = {ty: np.ascontiguousarray(
        np.asarray(inputs[f'x_{ty}'], np.float32)
        .reshape(cfg.n[ty], cfg.TF).astype(np.float16))
        for ty in ('node', 'pod', 'svc')}

    in_maps = []
    for c in range(cfg.ncores):
        in_maps.append({
            'x_node': xt['node'], 'x_pod': xt['pod'], 'x_svc': xt['svc'],
            'gidx': np.ascontiguousarray(gidx[c].T),
            's_all': s_all[c],
            'w_gc': wg, 'b_gc': bg, 'wx0': wx0, 'wr': wr, 'wl1': wl1,
            'b_lstm': bl,
        })

    # local row -> global output row map
    rowmap = np.full((cfg.ncores, cfg.nloc), -1, np.int64)
    gbase = {'node': 0, 'pod': cfg.n['node'], 'svc': cfg.n['node'] + cfg.n['pod']}
    for ty in ('node', 'pod', 'svc'):
        a, s = assign[ty], slot[ty]
        core = a // cfg.nb[ty]
        loc = (cfg.blk0[ty] + a % cfg.nb[ty]) * P + s
        rowmap[core, loc] = gbase[ty] + np.arange(cfg.n[ty])
    return K, qtot, in_maps, rowmap


def build_program(cfg, K, qtot):
    T, TF, TH, TO = cfg.T, cfg.TF, cfg.TH, cfg.TO
    nc = bacc.Bacc("TRN2", target_bir_lowering=False, debug=False,
                   num_devices=cfg.ncores)
    x = {ty: nc.dram_tensor(f"x_{ty}", [cfg.n[ty], TF], f16, kind="ExternalInput")
         for ty in ('node', 'pod', 'svc')}
    gidx = nc.dram_tensor("gidx", [P, qtot], i32, kind="ExternalInput")
    s_all = nc.dram_tensor("s_all", [qtot, P, P], f16, kind="ExternalInput")
    w_gc = nc.dram_tensor("w_gc", [P, 4 * cfg.H], f16, kind="ExternalInput")
    b_gc = nc.dram_tensor("b_gc", [P, 3], f32, kind="ExternalInput")
    wx0 = nc.dram_tensor("wx0", [P, 256], f16, kind="ExternalInput")
    wr = nc.dram_tensor("wr", [64, 256], f16, kind="ExternalInput")
    wl1 = nc.dram_tensor("wl1", [P, 256], f16, kind="ExternalInput")
    b_lstm = nc.dram_tensor("b_lstm", [P, 5], f32, kind="ExternalInput")
    out_feat = nc.dram_tensor("out_feat", [cfg.nloc, TH], f32, kind="ExternalOutput")
    out_h = nc.dram_tensor("out_h", [cfg.nloc, TO], f32, kind="ExternalOutput")

    NW = cfg.maxg * P  # LSTM free width (512)

    with tile.TileContext(nc) as tc:
        with (tc.tile_pool(name="const", bufs=1) as pc,
              tc.tile_pool(name="gio", bufs=4) as pg,
              tc.tile_pool(name="feat", bufs=3) as pw,
              tc.tile_pool(name="stage", bufs=2) as pw2,
              tc.tile_pool(name="mts", bufs=3) as pm,
              tc.tile_pool(name="state", bufs=2) as pst,
              tc.tile_pool(name="ew", bufs=4) as pl3,
              tc.tile_pool(name="htp", bufs=1, space="PSUM") as ppt,
              tc.tile_pool(name="gpsum", bufs=2, space="PSUM") as ppg,
              tc.tile_pool(name="mpsum", bufs=1, space="PSUM") as ppm,
              tc.tile_pool(name="pjp", bufs=1, space="PSUM") as ppj):
            idx_all = pc.tile([P, qtot], i32)
            nc.sync.dma_start(idx_all[:], gidx[:])
            wgc_t = pc.tile([P, 4 * cfg.H], f16)
            nc.sync.dma_start(wgc_t[:], w_gc[:])
            bgc_t = pc.tile([P, 3], f32)
            nc.sync.dma_start(bgc_t[:], b_gc[:])
            wx0_t = pc.tile([P, 256], f16)
            nc.sync.dma_start(wx0_t[:], wx0[:])
            wr_t = pc.tile([64, 256], f16)
            nc.sync.dma_start(wr_t[:], wr[:])
            wl1_t = pc.tile([P, 256], f16)
            nc.sync.dma_start(wl1_t[:], wl1[:])
            bl_t = pc.tile([P, 5], f32)
            nc.sync.dma_start(bl_t[:], b_lstm[:])
            ident = pc.tile([P, P], f16)
            make_identity(nc, ident[:])

        # ---- phase A+B for one block group: gather/aggregate/project ----
            qref = [0]

            def phase_ab(blks, featT):
                featT_v = featT[:].rearrange("p (t w) -> p t w", w=NW)
                for bi, b in enumerate(blks):
                    bt = cfg.block_type(b)
                    rels = TYPE_RELS[bt]
                    # mT targets: relation pairs stacked on partitions so the
                    # projection contracts both in one K=128 matmul
                    if bt == 'pod':
                        mtp = pm.tile([P, 2 * TF], f16, tag="mtp", name=f"mtp{b}")
                        mts1 = pm.tile([64, 2 * TF], f16, tag="mts", name=f"mts{b}")
                        targets = {'ni': (mtp, 0), 'ii': (mtp, 64), 'si': (mts1, 0)}
                        proj = [(mtp, 0, P), (mts1, P, 64)]
                    elif bt == 'svc':
                        mtp = pm.tile([P, 2 * TF], f16, tag="mtp", name=f"mtp{b}")
                        targets = {'sc': (mtp, 0), 'is': (mtp, 64)}
                        proj = [(mtp, 2 * P, P)]
                    else:
                        mts1 = pm.tile([64, 2 * TF], f16, tag="mts", name=f"mts{b}")
                        targets = {'in': (mts1, 0)}
                        proj = [(mts1, 3 * P, 64)]
                    for r in rels:
                        mpsum = ppm.tile([P, TF], f32, tag="mpsum")
                        Kbr = K[b][r]
                        for k in range(Kbr):
                            q = qref[0]
                            g = pg.tile([P, TF], f16, tag="g")
                            nc.gpsimd.indirect_dma_start(
                                out=g[:], out_offset=None,
                                in_=x[RELS[r][0]][:],
                                in_offset=bass.IndirectOffsetOnAxis(
                                    ap=idx_all[:, q:q + 1], axis=0))
                            st = pg.tile([P, P], f16, tag="s")
                            nc.sync.dma_start(st[:], s_all[q])
                            # psum "start" clears a whole 2KB bank: only
                            # bank-first slices start, bank-last slices stop
                            for j in range(TF // P):
                                nc.tensor.matmul(
                                    mpsum[:, j * P:(j + 1) * P],
                                    lhsT=g[:, j * P:(j + 1) * P], rhs=st[:],
                                    start=(k == 0 and j % 4 == 0),
                                    stop=(k == Kbr - 1 and j % 4 == 3))
                            qref[0] += 1
                        # de-interleave [t_even f | t_odd f] psum rows into the
                        # base-0 stacked tile (16-bit matmul operands at
                        # partition base 64 fault on HW)
                        tgt, row0 = targets[r]
                        tv = tgt[row0:row0 + 64, :].rearrange(
                            "p (t2 two d) -> p t2 two d", two=2, d=P)
                        mtmp = pg.tile([P, TF], f16, tag="mtmp")
                        nc.any.tensor_copy(mtmp[:], mpsum[:])
                        nc.sync.dma_start(
                            tv[:, :, 0, :],
                            mtmp[0:64, :].rearrange("p (j d) -> p j d", d=P))
                        nc.sync.dma_start(
                            tv[:, :, 1, :],
                            mtmp[64:128, :].rearrange("p (j d) -> p j d", d=P))
                    stf = pw2.tile([P, TH], f32, tag="stf")
                    ty = TYPE_IDX[bt]
                    for tq in range(T // 4):
                        pj = ppj.tile([P, 512], f32, tag="pj")
                        # mt columns are t-major so 4 timesteps project in one
                        # N=512 matmul per weight piece
                        for pi2, (mtile, wc, kk) in enumerate(proj):
                            nc.tensor.matmul(
                                pj[:],
                                lhsT=wgc_t[0:kk, wc:wc + P],
                                rhs=mtile[0:kk, tq * 512:(tq + 1) * 512],
                                start=(pi2 == 0),
                                stop=(pi2 == len(proj) - 1))
                        nc.scalar.activation(
                            featT_v[:, tq * 4:(tq + 1) * 4, bi * P:(bi + 1) * P],
                            pj[:].rearrange("p (t w) -> p t w", w=P),
                            AF.Relu, bias=bgc_t[:, ty:ty + 1],
                            scale=1.0 / len(rels))
                        f2 = ppj.tile([P, 512], f16, tag="pj")
                        for tt in range(4):
                            t = tq * 4 + tt
                            nc.tensor.transpose(
                                out=f2[:, tt * P:(tt + 1) * P],
                                in_=featT_v[:, t, bi * P:(bi + 1) * P],
                                identity=ident[:])
                        nc.any.tensor_copy(stf[:, tq * 512:(tq + 1) * 512],
                                           f2[:])
                    nc.sync.dma_start(out_feat[b * P:(b + 1) * P, :], stf[:])

            def lstm_pair_layer(pairst, t, layer, ga_pool, gb_pool):
                chains, hh, c0p, c1p, WT = pairst
                cur, nxt = hh[t % 2], hh[(t + 1) % 2]
                cp = c0p if layer == 0 else c1p
                bcol = 0 if layer == 0 else 2
                ga = ga_pool.tile([P, 2 * NW], f32, tag="gate",
                                  name=f"ga{t}_{layer}")
                gb = gb_pool.tile([P, 2 * NW], f32, tag="gate",
                                  name=f"gb{t}_{layer}")
                for ci, (featT, blks, sth) in enumerate(chains):
                    Wn = len(blks) * P
                    base = ci * NW
                    if layer == 0:
                        xin = featT[:, t * NW: t * NW + Wn]
                        hst = hh[(t + 1) % 2][0:64, base:base + Wn]
                        nc.tensor.matmul(ga[:, base:base + Wn],
                                         lhsT=wx0_t[:, 0:128], rhs=xin,
                                         start=True, stop=False)
                        nc.tensor.matmul(ga[:, base:base + Wn],
                                         lhsT=wr_t[:, 0:128], rhs=hst,
                                         start=False, stop=True)
                        nc.tensor.matmul(gb[:, base:base + Wn],
                                         lhsT=wx0_t[:, 128:256], rhs=xin,
                                         start=True, stop=False)
                        nc.tensor.matmul(gb[:, base:base + Wn],
                                         lhsT=wr_t[:, 128:256], rhs=hst,
                                         start=False, stop=True)
                    else:
                        # [h0(t); h1(t-1)] stacked: one K=128 matmul per tile
                        hst = cur[:, base:base + Wn]
                        nc.tensor.matmul(ga[:, base:base + Wn],
                                         lhsT=wl1_t[:, 0:128], rhs=hst,
                                         start=True, stop=True)
                        nc.tensor.matmul(gb[:, base:base + Wn],
                                         lhsT=wl1_t[:, 128:256], rhs=hst,
                                         start=True, stop=True)
                # paired elementwise: sif=[sig_f;sig_i], tgo=[tanh(o/2);tanh(g)]
                sifp = pl3.tile([P, 2 * NW], f16, tag="sif")
                nc.scalar.activation(sifp[:, :WT], ga[:, :WT], AF.Sigmoid,
                                     bias=bl_t[:, bcol:bcol + 1])
                tgop = pl3.tile([P, 2 * NW], f16, tag="tgo")
                nc.scalar.activation(tgop[:, :WT], gb[:, :WT], AF.Tanh,
                                     bias=bl_t[:, bcol + 1:bcol + 2],
                                     scale=bl_t[:, 4:5])
                prodG = pl3.tile([64, 2 * NW], f16, tag="prodG")
                nc.vector.tensor_mul(prodG[:, :WT], sifp[64:128, :WT],
                                     tgop[64:128, :WT])
                prodC = pl3.tile([64, 2 * NW], f16, tag="prodC")
                nc.vector.tensor_mul(prodC[:, :WT], sifp[0:64, :WT],
                                     cp[:, :WT])
                nc.vector.tensor_add(cp[:, :WT], prodG[:, :WT], prodC[:, :WT])
                tancp = pl3.tile([64, 2 * NW], f16, tag="tanc")
                nc.scalar.activation(tancp[:, :WT], cp[:, :WT], AF.Tanh)
                # h' = 2h = tanh(o/2)*tanh(c) + tanh(c); consumers carry the 0.5
                if layer == 0:
                    nc.vector.tensor_mul(cur[0:64, :WT], tgop[0:64, :WT],
                                         tancp[:, :WT])
                    nc.vector.tensor_add(cur[0:64, :WT], cur[0:64, :WT],
                                         tancp[:, :WT])
                    return None
                # layer1: build h in a base-0 temp (the stacked tile's h1 rows
                # sit at partition base 64, illegal as a 2-input DVE operand),
                # then copy into the next combined state tile
                htmp = pl3.tile([64, 2 * NW], f16, tag="prodG",
                                name=f"htmp{t}_{chains[0][1][0]}")
                nc.vector.tensor_mul(htmp[:, :WT], tgop[0:64, :WT],
                                     tancp[:, :WT])
                nc.vector.tensor_add(htmp[:, :WT], htmp[:, :WT],
                                     tancp[:, :WT])
                nc.vector.tensor_copy(nxt[64:128, :WT], htmp[:, :WT])
                return htmp

            def lstm_pair_hout(pairst, t, htmp):
                chains, hh, c0p, c1p, WT = pairst
                for ci, (featT, blks, sth) in enumerate(chains):
                    ng = len(blks)
                    ht = ppt.tile([P, ng * 64], f16, tag="ht")
                    for bi in range(ng):
                        c0 = ci * NW + bi * P
                        nc.tensor.transpose(
                            out=ht[:, bi * 64:(bi + 1) * 64],
                            in_=htmp[:, c0:c0 + P],
                            identity=ident[0:64, 0:64])
                    nc.scalar.activation(
                        sth[:].rearrange("p (m k) -> p m k", k=TO)[
                            :, 0:ng, t * 64:(t + 1) * 64],
                        ht[:].rearrange("p (m k) -> p m k", k=64),
                        AF.Relu, scale=0.5)

            # process chunks in pairs: build featT for both, then run the two
            # LSTM chains in lockstep, layers software-pipelined so L0(t+1)
            # overlaps L1(t).  The last pair's LSTM runs after all gather/
            # projection work, so its extra gate slot comes from the freed
            # mpsum/pj banks (scoped pools).
            def prep_pair(pair):
                chains = []
                for blks in pair:
                    featT = pw.tile([P, T * NW], f16, tag="featT",
                                    name=f"featT{blks[0]}")
                    phase_ab(blks, featT)
                    sth = pw2.tile([P, cfg.maxg * TO], f32, tag="sth",
                                   name=f"sth{blks[0]}")
                    chains.append((featT, blks, sth))
                return chains

            def run_pair(pair, chains, ga1_pool, gb1_pool):
                pid = pair[0][0]
                hh = [pst.tile([P, 2 * NW], f16, tag=f"hh{i}",
                               name=f"hh{i}_{pid}") for i in range(2)]
                c0p = pst.tile([64, 2 * NW], f16, tag="c0", name=f"c0_{pid}")
                c1p = pst.tile([64, 2 * NW], f16, tag="c1", name=f"c1_{pid}")
                for z in (hh[0], hh[1], c0p, c1p):
                    nc.gpsimd.memset(z[:], 0.0)
                WT = (len(pair) - 1) * NW + len(pair[-1]) * P
                ps = (chains, hh, c0p, c1p, WT)
                lstm_pair_layer(ps, 0, 0, ppg, ppg)
                for t in range(1, T):
                    lstm_pair_layer(ps, t, 0, ppg, ppg)
                    htmp = lstm_pair_layer(ps, t - 1, 1, ga1_pool, gb1_pool)
                    lstm_pair_hout(ps, t - 1, htmp)
                htmp = lstm_pair_layer(ps, T - 1, 1, ga1_pool, gb1_pool)
                lstm_pair_hout(ps, T - 1, htmp)
                for featT, blks, sth in chains:
                    for bi, b in enumerate(blks):
                        nc.sync.dma_start(out_h[b * P:(b + 1) * P, :],
                                          sth[:, bi * TO:(bi + 1) * TO])

            pairs = [cfg.chunks[i:i + 2] for i in range(0, len(cfg.chunks), 2)]
            for pair in pairs:
                run_pair(pair, prep_pair(pair), ppg, ppg)
            assert qref[0] == qtot
    nc.compile()
    return nc


_CACHE = {}


def _get_program(cfg, K, qtot):
    key = (cfg.nblk, cfg.T, cfg.ncores, qtot,
           tuple(tuple(sorted(kb.items())) for kb in K))
    if key not in _CACHE:
        _CACHE[key] = build_program(cfg, K, qtot)
    return _CACHE[key]


TRACE = False
_LAST = {}


def kernel(**inputs):
    cfg = CFG()
    K, qtot, in_maps, rowmap = preprocess(cfg, inputs)
    nc = _get_program(cfg, K, qtot)
    kw = {}
    if TRACE:
        kw = dict(trace=True)
    res = run_bass_kernel_spmd(nc, in_maps, core_ids=list(range(cfg.ncores)), **kw)
    _LAST['res'] = res

    n_total = sum(cfg.n.values())
    feat = np.zeros((n_total, cfg.T, cfg.H), np.float32)
    h = np.zeros((n_total, cfg.T, cfg.O), np.float32)
    for c in range(cfg.ncores):
        valid = rowmap[c] >= 0
        rows = rowmap[c][valid]
        feat[rows] = res.results[c]['out_feat'][valid].reshape(-1, cfg.T, cfg.H)
        h[rows] = res.results[c]['out_h'][valid].reshape(-1, cfg.T, cfg.O)
    return feat, h


# revision 24
# speedup vs baseline: 1.0028x; 1.0028x over previous
"""Trainium2 Bass kernel: hetero GraphConv (6 relations) + ReLU + 2-layer LSTM.

Strategy: shard destination nodes across 8 NeuronCores. Each core holds the
full source feature tables in HBM, gathers its incident edges' source rows
with indirect DMA, segment-sums them via one-hot scale-matrix matmuls into
PSUM, projects with the per-relation GraphConv weights (accumulated in PSUM,
ReLU+bias+mean fused on the scalar engine), then runs the 2-layer LSTM
data-parallel over its node rows.  No collectives are needed.

Data flows through the tensor engine in fp16 (features, scale matrices,
weights, LSTM states); all PSUM accumulation is fp32 and outputs are fp32.
"""
import sys

sys.path.insert(0, '/opt/trn_rl_repo')
import numpy as np

from concourse import bass, mybir, tile, bacc
from concourse.bass_utils import run_bass_kernel_spmd
from concourse.masks import make_identity

f32 = mybir.dt.float32
f16 = mybir.dt.float16
i32 = mybir.dt.int32
AF = mybir.ActivationFunctionType

P = 128

# relation -> (src type, dst type, weight column index)
RELS = {
    'in': ('pod', 'node', 0),
    'ni': ('node', 'pod', 1),
    'ii': ('pod', 'pod', 2),
    'si': ('svc', 'pod', 3),
    'sc': ('svc', 'svc', 4),
    'is': ('pod', 'svc', 5),
}
TYPE_RELS = {'node': ['in'], 'pod': ['ni', 'ii', 'si'], 'svc': ['sc', 'is']}
TYPE_IDX = {'node': 0, 'pod': 1, 'svc': 2}


class CFG:
    def __init__(self, n_node=500, n_pod=20000, n_svc=2000, T=16,
                 ncores=8, nodb=1, podb=20, svcb=2, lstm_grp=4):
        self.n = {'node': n_node, 'pod': n_pod, 'svc': n_svc}
        self.T = T
        self.F, self.H, self.O = 64, 128, 64
        self.TF, self.TH, self.TO = T * 64, T * 128, T * 64
        self.ncores = ncores
        self.nb = {'node': nodb, 'pod': podb, 'svc': svcb}
        # pods first and the gather-heavy node block last, so the first LSTM
        # pair's inputs are ready quickly and the node block's long gather
        # burst overlaps mid-kernel LSTM work
        self.blk0 = {'pod': 0, 'svc': podb, 'node': podb + svcb}
        self.nblk = nodb + podb + svcb
        self.nloc = self.nblk * P
        self.slot_cap = {ty: min(P, -(-self.n[ty] // (ncores * self.nb[ty])))
                         for ty in self.n}
        self.chunks = []
        b = 0
        while b < self.nblk:
            self.chunks.append(list(range(b, min(b + lstm_grp, self.nblk))))
            b += lstm_grp
        self.maxg = max(len(c) for c in self.chunks)

    def block_type(self, b):
        if b < self.blk0['svc']:
            return 'pod'
        if b < self.blk0['node']:
            return 'svc'
        return 'node'


def _pack_bins(degs, n_bins, slot_cap):
    """Greedy multi-constraint balancing of items (rows of degs) into bins."""
    n_items, R = degs.shape
    caps = np.maximum(degs.sum(0) / n_bins, 1.0)
    order = np.argsort(-(degs / caps).sum(1), kind='stable')
    bin_cnt = np.zeros((n_bins, R))
    bin_slots = np.zeros(n_bins, np.int64)
    assign = np.empty(n_items, np.int64)
    for i in order:
        load = ((bin_cnt + degs[i]) / caps).max(1) + bin_slots * 1e-5
        if (bin_slots >= slot_cap).any():
            load = np.where(bin_slots >= slot_cap, np.inf, load)
        b = int(np.argmin(load))
        assign[i] = b
        bin_cnt[b] += degs[i]
        bin_slots[b] += 1
    return assign


def preprocess(cfg, inputs):
    """Host-side graph partitioning. Returns per-core input maps + metadata."""
    edges = {}
    for r in RELS:
        src = np.asarray(inputs[f'{r}_src']).astype(np.int64)
        dst = np.asarray(inputs[f'{r}_dst']).astype(np.int64)
        edges[r] = (src, dst)

    # per-edge normalization scale (DGL norm='both')
    scale = {}
    for r, (sk, dk, _) in RELS.items():
        src, dst = edges[r]
        outd = np.maximum(np.bincount(src, minlength=cfg.n[sk]), 1).astype(np.float32)
        ind = np.maximum(np.bincount(dst, minlength=cfg.n[dk]), 1).astype(np.float32)
        rs_o = (1.0 / np.sqrt(outd)).astype(np.float32)
        rs_i = (1.0 / np.sqrt(ind)).astype(np.float32)
        scale[r] = (rs_o[src] * rs_i[dst]).astype(np.float32)

    # pack dst nodes of each type into (core, block) bins, balancing per-relation
    # in-degree so per-block chunk counts stay uniform
    assign, slot = {}, {}
    for ty in ('node', 'pod', 'svc'):
        rels = TYPE_RELS[ty]
        degs = np.stack(
            [np.bincount(edges[r][1], minlength=cfg.n[ty]) for r in rels], axis=1
        ).astype(np.float64)
        n_bins = cfg.ncores * cfg.nb[ty]
        a = _pack_bins(degs, n_bins, cfg.slot_cap[ty])
        assign[ty] = a
        s = np.zeros(cfg.n[ty], np.int64)
        for b in range(n_bins):
            ids = np.where(a == b)[0]
            s[ids] = np.arange(len(ids))
        slot[ty] = s

    # per (core, block, rel) edge lists; chunk counts maxed over cores
    by_cbr = {}
    for r, (sk, dk, _) in RELS.items():
        src, dst = edges[r]
        bins = assign[dk][dst]
        nbc = cfg.nb[dk]
        core = bins // nbc
        blk = cfg.blk0[dk] + bins % nbc
        sl = slot[dk][dst]
        order = np.lexsort((sl, blk, core))
        src, core, blk, sl, sc = (src[order], core[order], blk[order],
                                  sl[order], scale[r][order])
        for c in range(cfg.ncores):
            m = core == c
            by_cbr.setdefault(c, {})
            for b in range(cfg.nblk):
                mb = m & (blk == b)
                if not mb.any():
                    continue
                by_cbr[c].setdefault(b, {})[r] = (src[mb], sl[mb], sc[mb])

    K = []
    for b in range(cfg.nblk):
        bt = cfg.block_type(b)
        kb = {}
        for r in TYPE_RELS[bt]:
            mx = 1
            for c in range(cfg.ncores):
                e = by_cbr.get(c, {}).get(b, {}).get(r)
                if e is not None:
                    mx = max(mx, -(-len(e[0]) // P))
            kb[r] = mx
        K.append(kb)
    qtot = sum(sum(kb.values()) for kb in K)

    # flatten per-core gather indices + scale matrices in program order
    gidx = np.zeros((cfg.ncores, qtot, P), np.int32)
    s_all = np.zeros((cfg.ncores, qtot, P, P), np.float16)
    for c in range(cfg.ncores):
        q = 0
        for b in range(cfg.nblk):
            bt = cfg.block_type(b)
            for r in TYPE_RELS[bt]:
                e = by_cbr.get(c, {}).get(b, {}).get(r)
                if e is not None:
                    es, el, ev = e
                    n = len(es)
                    ch = q + np.arange(n) // P
                    ro = np.arange(n) % P
                    gidx[c, ch, ro] = es
                    s_all[c, ch, ro, el] = ev
                q += K[b][r]
        assert q == qtot

    # stacked projection weights: col block 0 = [W_ni;W_ii], 1 = [W_si;0],
    # 2 = [W_sc;W_is], 3 = [W_in;0] -- pairs contract as one K=128 matmul
    wg = np.zeros((128, 4 * cfg.H), np.float16)
    WI = {r: np.asarray(inputs[f'W_{r}'], np.float32) for r in RELS}
    wg[0:64, 0:128] = WI['ni']
    wg[64:128, 0:128] = WI['ii']
    wg[0:64, 128:256] = WI['si']
    wg[0:64, 256:384] = WI['sc']
    wg[64:128, 256:384] = WI['is']
    wg[0:64, 384:512] = WI['in']
    bg = np.zeros((P, 3), np.float32)
    for ty, rels in TYPE_RELS.items():
        bsum = sum(np.asarray(inputs[f'b_{r}'], np.float32) for r in rels)
        bg[:, TYPE_IDX[ty]] = bsum / len(rels)

    # LSTM gate rows reordered i,f,g,o -> f,i,o,g.  sigmoid(o) is computed as
    # tanh(o/2) alongside tanh(g) in one scale-vectored ACT op; the missing
    # (x+1)/2 affine is recovered by computing h' = 2h and folding 0.5 into
    # every consumer of h (Whh0, Wih1, Whh1 and the output ReLU scale).
    perm = np.r_[64:128, 0:64, 192:256, 128:192]
    wx0 = np.ascontiguousarray(
        np.asarray(inputs['Wih0'], np.float32).T[:, perm]).astype(np.float16)
    wr = np.ascontiguousarray(
        np.asarray(inputs['Whh0'], np.float32).T[:, perm] * 0.5
    ).astype(np.float16)  # [O, 4O]
    # layer1 x-input (h0) and recurrent (h1) weights stacked on K so both
    # contract in one K=128 matmul against the combined [h0;h1] state tile
    wl1 = np.ascontiguousarray(np.vstack(
        [np.asarray(inputs['Wih1'], np.float32).T[:, perm] * 0.5,
         np.asarray(inputs['Whh1'], np.float32).T[:, perm] * 0.5]
    )).astype(np.float16)  # [2O, 4O]
    b0 = (np.asarray(inputs['bih0'], np.float32)
          + np.asarray(inputs['bhh0'], np.float32))[perm]
    b1 = (np.asarray(inputs['bih1'], np.float32)
          + np.asarray(inputs['bhh1'], np.float32))[perm]
    b0[128:192] *= 0.5  # o-gate bias halved (tanh(x/2) trick)
    b1[128:192] *= 0.5
    tsc = np.concatenate([np.full(64, 0.5), np.ones(64)]).astype(np.float32)
    bl = np.stack([b0[:P], b0[P:], b1[:P], b1[P:], tsc], axis=1).astype(np.float32)

    xt = {ty: np.ascontiguousarray(
        np.asarray(inputs[f'x_{ty}'], np.float32)
        .reshape(cfg.n[ty], cfg.TF).astype(np.float16))
        for ty in ('node', 'pod', 'svc')}

    in_maps = []
    for c in range(cfg.ncores):
        in_maps.append({
            'x_node': xt['node'], 'x_pod': xt['pod'], 'x_svc': xt['svc'],
            'gidx': np.ascontiguousarray(gidx[c].T),
            's_all': s_all[c],
            'w_gc': wg, 'b_gc': bg, 'wx0': wx0, 'wr': wr, 'wl1': wl1,
            'b_lstm': bl,
        })

    # local row -> global output row map
    rowmap = np.full((cfg.ncores, cfg.nloc), -1, np.int64)
    gbase = {'node': 0, 'pod': cfg.n['node'], 'svc': cfg.n['node'] + cfg.n['pod']}
    for ty in ('node', 'pod', 'svc'):
        a, s = assign[ty], slot[ty]
        core = a // cfg.nb[ty]
        loc = (cfg.blk0[ty] + a % cfg.nb[ty]) * P + s
        rowmap[core, loc] = gbase[ty] + np.arange(cfg.n[ty])
    return K, qtot, in_maps, rowmap


def build_program(cfg, K, qtot):
    T, TF, TH, TO = cfg.T, cfg.TF, cfg.TH, cfg.TO
    nc = bacc.Bacc("TRN2", target_bir_lowering=False, debug=False,
                   num_devices=cfg.ncores)
    x = {ty: nc.dram_tensor(f"x_{ty}", [cfg.n[ty], TF], f16, kind="ExternalInput")
         for ty in ('node', 'pod', 'svc')}
    gidx = nc.dram_tensor("gidx", [P, qtot], i32, kind="ExternalInput")
    s_all = nc.dram_tensor("s_all", [qtot, P, P], f16, kind="ExternalInput")
    w_gc = nc.dram_tensor("w_gc", [P, 4 * cfg.H], f16, kind="ExternalInput")
    b_gc = nc.dram_tensor("b_gc", [P, 3], f32, kind="ExternalInput")
    wx0 = nc.dram_tensor("wx0", [P, 256], f16, kind="ExternalInput")
    wr = nc.dram_tensor("wr", [64, 256], f16, kind="ExternalInput")
    wl1 = nc.dram_tensor("wl1", [P, 256], f16, kind="ExternalInput")
    b_lstm = nc.dram_tensor("b_lstm", [P, 5], f32, kind="ExternalInput")
    out_feat = nc.dram_tensor("out_feat", [cfg.nloc, TH], f32, kind="ExternalOutput")
    out_h = nc.dram_tensor("out_h", [cfg.nloc, TO], f32, kind="ExternalOutput")

    NW = cfg.maxg * P  # LSTM free width (512)

    with tile.TileContext(nc) as tc:
        with (tc.tile_pool(name="const", bufs=1) as pc,
              tc.tile_pool(name="gio", bufs=5) as pg,
              tc.tile_pool(name="feat", bufs=3) as pw,
              tc.tile_pool(name="stage", bufs=2) as pw2,
              tc.tile_pool(name="mts", bufs=3) as pm,
              tc.tile_pool(name="state", bufs=2) as pst,
              tc.tile_pool(name="ew", bufs=4) as pl3,
              tc.tile_pool(name="htp", bufs=1, space="PSUM") as ppt,
              tc.tile_pool(name="gpsum", bufs=2, space="PSUM") as ppg,
              tc.tile_pool(name="mpsum", bufs=1, space="PSUM") as ppm,
              tc.tile_pool(name="pjp", bufs=1, space="PSUM") as ppj):
            idx_all = pc.tile([P, qtot], i32)
            nc.sync.dma_start(idx_all[:], gidx[:])
            wgc_t = pc.tile([P, 4 * cfg.H], f16)
            nc.sync.dma_start(wgc_t[:], w_gc[:])
            bgc_t = pc.tile([P, 3], f32)
            nc.sync.dma_start(bgc_t[:], b_gc[:])
            wx0_t = pc.tile([P, 256], f16)
            nc.sync.dma_start(wx0_t[:], wx0[:])
            wr_t = pc.tile([64, 256], f16)
            nc.sync.dma_start(wr_t[:], wr[:])
            wl1_t = pc.tile([P, 256], f16)
            nc.sync.dma_start(wl1_t[:], wl1[:])
            bl_t = pc.tile([P, 5], f32)
            nc.sync.dma_start(bl_t[:], b_lstm[:])
            ident = pc.tile([P, P], f16)
            make_identity(nc, ident[:])

        # ---- phase A+B for one block group: gather/aggregate/project ----
            qref = [0]

            def phase_ab(blks, featT):
                featT_v = featT[:].rearrange("p (t w) -> p t w", w=NW)
                for bi, b in enumerate(blks):
                    bt = cfg.block_type(b)
                    rels = TYPE_RELS[bt]
                    # mT targets: relation pairs stacked on partitions so the
                    # projection contracts both in one K=128 matmul
                    if bt == 'pod':
                        mtp = pm.tile([P, 2 * TF], f16, tag="mtp", name=f"mtp{b}")
                        mts1 = pm.tile([64, 2 * TF], f16, tag="mts", name=f"mts{b}")
                        targets = {'ni': (mtp, 0), 'ii': (mtp, 64), 'si': (mts1, 0)}
                        proj = [(mtp, 0, P), (mts1, P, 64)]
                    elif bt == 'svc':
                        mtp = pm.tile([P, 2 * TF], f16, tag="mtp", name=f"mtp{b}")
                        targets = {'sc': (mtp, 0), 'is': (mtp, 64)}
                        proj = [(mtp, 2 * P, P)]
                    else:
                        mts1 = pm.tile([64, 2 * TF], f16, tag="mts", name=f"mts{b}")
                        targets = {'in': (mts1, 0)}
                        proj = [(mts1, 3 * P, 64)]
                    for r in rels:
                        mpsum = ppm.tile([P, TF], f32, tag="mpsum")
                        Kbr = K[b][r]
                        for k in range(Kbr):
                            q = qref[0]
                            g = pg.tile([P, TF], f16, tag="g")
                            nc.gpsimd.indirect_dma_start(
                                out=g[:], out_offset=None,
                                in_=x[RELS[r][0]][:],
                                in_offset=bass.IndirectOffsetOnAxis(
                                    ap=idx_all[:, q:q + 1], axis=0))
                            st = pg.tile([P, P], f16, tag="s")
                            nc.sync.dma_start(st[:], s_all[q])
                            # psum "start" clears a whole 2KB bank: only
                            # bank-first slices start, bank-last slices stop
                            for j in range(TF // P):
                                nc.tensor.matmul(
                                    mpsum[:, j * P:(j + 1) * P],
                                    lhsT=g[:, j * P:(j + 1) * P], rhs=st[:],
                                    start=(k == 0 and j % 4 == 0),
                                    stop=(k == Kbr - 1 and j % 4 == 3))
                            qref[0] += 1
                        # de-interleave [t_even f | t_odd f] psum rows into the
                        # base-0 stacked tile (16-bit matmul operands at
                        # partition base 64 fault on HW)
                        tgt, row0 = targets[r]
                        tv = tgt[row0:row0 + 64, :].rearrange(
                            "p (t2 two d) -> p t2 two d", two=2, d=P)
                        mtmp = pg.tile([P, TF], f16, tag="mtmp")
                        nc.vector.tensor_copy(mtmp[:], mpsum[:])
                        nc.sync.dma_start(
                            tv[:, :, 0, :],
                            mtmp[0:64, :].rearrange("p (j d) -> p j d", d=P))
                        nc.sync.dma_start(
                            tv[:, :, 1, :],
                            mtmp[64:128, :].rearrange("p (j d) -> p j d", d=P))
                    stf = pw2.tile([P, TH], f32, tag="stf")
                    ty = TYPE_IDX[bt]
                    for tq in range(T // 4):
                        pj = ppj.tile([P, 512], f32, tag="pj")
                        # mt columns are t-major so 4 timesteps project in one
                        # N=512 matmul per weight piece
                        for pi2, (mtile, wc, kk) in enumerate(proj):
                            nc.tensor.matmul(
                                pj[:],
                                lhsT=wgc_t[0:kk, wc:wc + P],
                                rhs=mtile[0:kk, tq * 512:(tq + 1) * 512],
                                start=(pi2 == 0),
                                stop=(pi2 == len(proj) - 1))
                        nc.scalar.activation(
                            featT_v[:, tq * 4:(tq + 1) * 4, bi * P:(bi + 1) * P],
                            pj[:].rearrange("p (t w) -> p t w", w=P),
                            AF.Relu, bias=bgc_t[:, ty:ty + 1],
                            scale=1.0 / len(rels))
                        f2 = ppj.tile([P, 512], f16, tag="pj")
                        for tt in range(4):
                            t = tq * 4 + tt
                            nc.tensor.transpose(
                                out=f2[:, tt * P:(tt + 1) * P],
                                in_=featT_v[:, t, bi * P:(bi + 1) * P],
                                identity=ident[:])
                        nc.vector.tensor_copy(stf[:, tq * 512:(tq + 1) * 512],
                                              f2[:])
                    nc.sync.dma_start(out_feat[b * P:(b + 1) * P, :], stf[:])

            def lstm_pair_layer(pairst, t, layer, ga_pool, gb_pool):
                chains, hh, c0p, c1p, WT = pairst
                cur, nxt = hh[t % 2], hh[(t + 1) % 2]
                cp = c0p if layer == 0 else c1p
                bcol = 0 if layer == 0 else 2
                ga = ga_pool.tile([P, 2 * NW], f32, tag="gate",
                                  name=f"ga{t}_{layer}")
                gb = gb_pool.tile([P, 2 * NW], f32, tag="gate",
                                  name=f"gb{t}_{layer}")
                for ci, (featT, blks, sth) in enumerate(chains):
                    Wn = len(blks) * P
                    base = ci * NW
                    if layer == 0:
                        xin = featT[:, t * NW: t * NW + Wn]
                        hst = hh[(t + 1) % 2][0:64, base:base + Wn]
                        nc.tensor.matmul(ga[:, base:base + Wn],
                                         lhsT=wx0_t[:, 0:128], rhs=xin,
                                         start=True, stop=False)
                        nc.tensor.matmul(ga[:, base:base + Wn],
                                         lhsT=wr_t[:, 0:128], rhs=hst,
                                         start=False, stop=True)
                        nc.tensor.matmul(gb[:, base:base + Wn],
                                         lhsT=wx0_t[:, 128:256], rhs=xin,
                                         start=True, stop=False)
                        nc.tensor.matmul(gb[:, base:base + Wn],
                                         lhsT=wr_t[:, 128:256], rhs=hst,
                                         start=False, stop=True)
                    else:
                        # [h0(t); h1(t-1)] stacked: one K=128 matmul per tile
                        hst = cur[:, base:base + Wn]
                        nc.tensor.matmul(ga[:, base:base + Wn],
                                         lhsT=wl1_t[:, 0:128], rhs=hst,
                                         start=True, stop=True)
                        nc.tensor.matmul(gb[:, base:base + Wn],
                                         lhsT=wl1_t[:, 128:256], rhs=hst,
                                         start=True, stop=True)
                # paired elementwise: sif=[sig_f;sig_i], tgo=[tanh(o/2);tanh(g)]
                sifp = pl3.tile([P, 2 * NW], f16, tag="sif")
                nc.scalar.activation(sifp[:, :WT], ga[:, :WT], AF.Sigmoid,
                                     bias=bl_t[:, bcol:bcol + 1])
                tgop = pl3.tile([P, 2 * NW], f16, tag="tgo")
                nc.scalar.activation(tgop[:, :WT], gb[:, :WT], AF.Tanh,
                                     bias=bl_t[:, bcol + 1:bcol + 2],
                                     scale=bl_t[:, 4:5])
                prodG = pl3.tile([64, 2 * NW], f16, tag="prodG")
                nc.vector.tensor_mul(prodG[:, :WT], sifp[64:128, :WT],
                                     tgop[64:128, :WT])
                prodC = pl3.tile([64, 2 * NW], f16, tag="prodC")
                nc.vector.tensor_mul(prodC[:, :WT], sifp[0:64, :WT],
                                     cp[:, :WT])
                nc.vector.tensor_add(cp[:, :WT], prodG[:, :WT], prodC[:, :WT])
                tancp = pl3.tile([64, 2 * NW], f16, tag="tanc")
                nc.scalar.activation(tancp[:, :WT], cp[:, :WT], AF.Tanh)
                # h' = 2h = tanh(o/2)*tanh(c) + tanh(c); consumers carry the 0.5
                if layer == 0:
                    nc.vector.tensor_mul(cur[0:64, :WT], tgop[0:64, :WT],
                                         tancp[:, :WT])
                    nc.vector.tensor_add(cur[0:64, :WT], cur[0:64, :WT],
                                         tancp[:, :WT])
                    return None
                # layer1: build h in a base-0 temp (the stacked tile's h1 rows
                # sit at partition base 64, illegal as a 2-input DVE operand),
                # then copy into the next combined state tile
                htmp = pl3.tile([64, 2 * NW], f16, tag="prodG",
                                name=f"htmp{t}_{chains[0][1][0]}")
                nc.vector.tensor_mul(htmp[:, :WT], tgop[0:64, :WT],
                                     tancp[:, :WT])
                nc.vector.tensor_add(htmp[:, :WT], htmp[:, :WT],
                                     tancp[:, :WT])
                nc.vector.tensor_copy(nxt[64:128, :WT], htmp[:, :WT])
                return htmp

            def lstm_pair_hout(pairst, t, htmp):
                chains, hh, c0p, c1p, WT = pairst
                for ci, (featT, blks, sth) in enumerate(chains):
                    ng = len(blks)
                    ht = ppt.tile([P, ng * 64], f16, tag="ht")
                    for bi in range(ng):
                        c0 = ci * NW + bi * P
                        nc.tensor.transpose(
                            out=ht[:, bi * 64:(bi + 1) * 64],
                            in_=htmp[:, c0:c0 + P],
                            identity=ident[0:64, 0:64])
                    nc.scalar.activation(
                        sth[:].rearrange("p (m k) -> p m k", k=TO)[
                            :, 0:ng, t * 64:(t + 1) * 64],
                        ht[:].rearrange("p (m k) -> p m k", k=64),
                        AF.Relu, scale=0.5)

            # process chunks in pairs: build featT for both, then run the two
            # LSTM chains in lockstep, layers software-pipelined so L0(t+1)
            # overlaps L1(t).  The last pair's LSTM runs after all gather/
            # projection work, so its extra gate slot comes from the freed
            # mpsum/pj banks (scoped pools).
            def prep_pair(pair):
                chains = []
                for blks in pair:
                    featT = pw.tile([P, T * NW], f16, tag="featT",
                                    name=f"featT{blks[0]}")
                    phase_ab(blks, featT)
                    sth = pw2.tile([P, cfg.maxg * TO], f32, tag="sth",
                                   name=f"sth{blks[0]}")
                    chains.append((featT, blks, sth))
                return chains

            def run_pair(pair, chains, ga1_pool, gb1_pool):
                pid = pair[0][0]
                hh = [pst.tile([P, 2 * NW], f16, tag=f"hh{i}",
                               name=f"hh{i}_{pid}") for i in range(2)]
                c0p = pst.tile([64, 2 * NW], f16, tag="c0", name=f"c0_{pid}")
                c1p = pst.tile([64, 2 * NW], f16, tag="c1", name=f"c1_{pid}")
                for z in (hh[0], hh[1], c0p, c1p):
                    nc.gpsimd.memset(z[:], 0.0)
                WT = (len(pair) - 1) * NW + len(pair[-1]) * P
                ps = (chains, hh, c0p, c1p, WT)
                lstm_pair_layer(ps, 0, 0, ppg, ppg)
                for t in range(1, T):
                    lstm_pair_layer(ps, t, 0, ppg, ppg)
                    htmp = lstm_pair_layer(ps, t - 1, 1, ga1_pool, gb1_pool)
                    lstm_pair_hout(ps, t - 1, htmp)
                htmp = lstm_pair_layer(ps, T - 1, 1, ga1_pool, gb1_pool)
                lstm_pair_hout(ps, T - 1, htmp)
                for featT, blks, sth in chains:
                    for bi, b in enumerate(blks):
                        nc.sync.dma_start(out_h[b * P:(b + 1) * P, :],
                                          sth[:, bi * TO:(bi + 1) * TO])

            pairs = [cfg.chunks[i:i + 2] for i in range(0, len(cfg.chunks), 2)]
            for pair in pairs:
                run_pair(pair, prep_pair(pair), ppg, ppg)
            assert qref[0] == qtot
    nc.compile()
    return nc


_CACHE = {}


def _get_program(cfg, K, qtot):
    key = (cfg.nblk, cfg.T, cfg.ncores, qtot,
           tuple(tuple(sorted(kb.items())) for kb in K))
    if key not in _CACHE:
        _CACHE[key] = build_program(cfg, K, qtot)
    return _CACHE[key]


TRACE = False
_LAST = {}


def kernel(**inputs):
    cfg = CFG()
    K, qtot, in_maps, rowmap = preprocess(cfg, inputs)
    nc = _get_program(cfg, K, qtot)
    kw = {}
    if TRACE:
        kw = dict(trace=True)
    res = run_bass_kernel_spmd(nc, in_maps, core_ids=list(range(cfg.ncores)), **kw)
    _LAST['res'] = res

    n_total = sum(cfg.n.values())
    feat = np.zeros((n_total, cfg.T, cfg.H), np.float32)
    h = np.zeros((n_total, cfg.T, cfg.O), np.float32)
    for c in range(cfg.ncores):
        valid = rowmap[c] >= 0
        rows = rowmap[c][valid]
        feat[rows] = res.results[c]['out_feat'][valid].reshape(-1, cfg.T, cfg.H)
        h[rows] = res.results[c]['out_h'][valid].reshape(-1, cfg.T, cfg.O)
    return feat, h


# revision 25
# speedup vs baseline: 1.0090x; 1.0062x over previous
"""Trainium2 Bass kernel: hetero GraphConv (6 relations) + ReLU + 2-layer LSTM.

Strategy: shard destination nodes across 8 NeuronCores. Each core holds the
full source feature tables in HBM, gathers its incident edges' source rows
with indirect DMA, segment-sums them via one-hot scale-matrix matmuls into
PSUM, projects with the per-relation GraphConv weights (accumulated in PSUM,
ReLU+bias+mean fused on the scalar engine), then runs the 2-layer LSTM
data-parallel over its node rows.  No collectives are needed.

Data flows through the tensor engine in fp16 (features, scale matrices,
weights, LSTM states); all PSUM accumulation is fp32 and outputs are fp32.
"""
import sys

sys.path.insert(0, '/opt/trn_rl_repo')
import numpy as np

from concourse import bass, mybir, tile, bacc
from concourse.bass_utils import run_bass_kernel_spmd
from concourse.masks import make_identity

f32 = mybir.dt.float32
f16 = mybir.dt.float16
i32 = mybir.dt.int32
AF = mybir.ActivationFunctionType

P = 128

# relation -> (src type, dst type, weight column index)
RELS = {
    'in': ('pod', 'node', 0),
    'ni': ('node', 'pod', 1),
    'ii': ('pod', 'pod', 2),
    'si': ('svc', 'pod', 3),
    'sc': ('svc', 'svc', 4),
    'is': ('pod', 'svc', 5),
}
TYPE_RELS = {'node': ['in'], 'pod': ['ni', 'ii', 'si'], 'svc': ['sc', 'is']}
TYPE_IDX = {'node': 0, 'pod': 1, 'svc': 2}


class CFG:
    def __init__(self, n_node=500, n_pod=20000, n_svc=2000, T=16,
                 ncores=8, nodb=1, podb=20, svcb=2, lstm_grp=4):
        self.n = {'node': n_node, 'pod': n_pod, 'svc': n_svc}
        self.T = T
        self.F, self.H, self.O = 64, 128, 64
        self.TF, self.TH, self.TO = T * 64, T * 128, T * 64
        self.ncores = ncores
        self.nb = {'node': nodb, 'pod': podb, 'svc': svcb}
        # pods first and the gather-heavy node block last, so the first LSTM
        # pair's inputs are ready quickly and the node block's long gather
        # burst overlaps mid-kernel LSTM work
        self.blk0 = {'pod': 0, 'svc': podb, 'node': podb + svcb}
        self.nblk = nodb + podb + svcb
        self.nloc = self.nblk * P
        self.slot_cap = {ty: min(P, -(-self.n[ty] // (ncores * self.nb[ty])))
                         for ty in self.n}
        self.chunks = []
        b = 0
        while b < self.nblk:
            self.chunks.append(list(range(b, min(b + lstm_grp, self.nblk))))
            b += lstm_grp
        self.maxg = max(len(c) for c in self.chunks)

    def block_type(self, b):
        if b < self.blk0['svc']:
            return 'pod'
        if b < self.blk0['node']:
            return 'svc'
        return 'node'


def _pack_bins(degs, n_bins, slot_cap):
    """Greedy multi-constraint balancing of items (rows of degs) into bins."""
    n_items, R = degs.shape
    caps = np.maximum(degs.sum(0) / n_bins, 1.0)
    order = np.argsort(-(degs / caps).sum(1), kind='stable')
    bin_cnt = np.zeros((n_bins, R))
    bin_slots = np.zeros(n_bins, np.int64)
    assign = np.empty(n_items, np.int64)
    for i in order:
        load = ((bin_cnt + degs[i]) / caps).max(1) + bin_slots * 1e-5
        if (bin_slots >= slot_cap).any():
            load = np.where(bin_slots >= slot_cap, np.inf, load)
        b = int(np.argmin(load))
        assign[i] = b
        bin_cnt[b] += degs[i]
        bin_slots[b] += 1
    return assign


def preprocess(cfg, inputs):
    """Host-side graph partitioning. Returns per-core input maps + metadata."""
    edges = {}
    for r in RELS:
        src = np.asarray(inputs[f'{r}_src']).astype(np.int64)
        dst = np.asarray(inputs[f'{r}_dst']).astype(np.int64)
        edges[r] = (src, dst)

    # per-edge normalization scale (DGL norm='both')
    scale = {}
    for r, (sk, dk, _) in RELS.items():
        src, dst = edges[r]
        outd = np.maximum(np.bincount(src, minlength=cfg.n[sk]), 1).astype(np.float32)
        ind = np.maximum(np.bincount(dst, minlength=cfg.n[dk]), 1).astype(np.float32)
        rs_o = (1.0 / np.sqrt(outd)).astype(np.float32)
        rs_i = (1.0 / np.sqrt(ind)).astype(np.float32)
        scale[r] = (rs_o[src] * rs_i[dst]).astype(np.float32)

    # pack dst nodes of each type into (core, block) bins, balancing per-relation
    # in-degree so per-block chunk counts stay uniform
    assign, slot = {}, {}
    for ty in ('node', 'pod', 'svc'):
        rels = TYPE_RELS[ty]
        degs = np.stack(
            [np.bincount(edges[r][1], minlength=cfg.n[ty]) for r in rels], axis=1
        ).astype(np.float64)
        n_bins = cfg.ncores * cfg.nb[ty]
        a = _pack_bins(degs, n_bins, cfg.slot_cap[ty])
        assign[ty] = a
        s = np.zeros(cfg.n[ty], np.int64)
        for b in range(n_bins):
            ids = np.where(a == b)[0]
            s[ids] = np.arange(len(ids))
        slot[ty] = s

    # per (core, block, rel) edge lists; chunk counts maxed over cores
    by_cbr = {}
    for r, (sk, dk, _) in RELS.items():
        src, dst = edges[r]
        bins = assign[dk][dst]
        nbc = cfg.nb[dk]
        core = bins // nbc
        blk = cfg.blk0[dk] + bins % nbc
        sl = slot[dk][dst]
        order = np.lexsort((sl, blk, core))
        src, core, blk, sl, sc = (src[order], core[order], blk[order],
                                  sl[order], scale[r][order])
        for c in range(cfg.ncores):
            m = core == c
            by_cbr.setdefault(c, {})
            for b in range(cfg.nblk):
                mb = m & (blk == b)
                if not mb.any():
                    continue
                by_cbr[c].setdefault(b, {})[r] = (src[mb], sl[mb], sc[mb])

    K = []
    for b in range(cfg.nblk):
        bt = cfg.block_type(b)
        kb = {}
        for r in TYPE_RELS[bt]:
            mx = 1
            for c in range(cfg.ncores):
                e = by_cbr.get(c, {}).get(b, {}).get(r)
                if e is not None:
                    mx = max(mx, -(-len(e[0]) // P))
            kb[r] = mx
        K.append(kb)
    qtot = sum(sum(kb.values()) for kb in K)

    # flatten per-core gather indices + scale matrices in program order
    gidx = np.zeros((cfg.ncores, qtot, P), np.int32)
    s_all = np.zeros((cfg.ncores, qtot, P, P), np.float16)
    for c in range(cfg.ncores):
        q = 0
        for b in range(cfg.nblk):
            bt = cfg.block_type(b)
            for r in TYPE_RELS[bt]:
                e = by_cbr.get(c, {}).get(b, {}).get(r)
                if e is not None:
                    es, el, ev = e
                    n = len(es)
                    ch = q + np.arange(n) // P
                    ro = np.arange(n) % P
                    gidx[c, ch, ro] = es
                    s_all[c, ch, ro, el] = ev
                q += K[b][r]
        assert q == qtot

    # stacked projection weights: col block 0 = [W_ni;W_ii], 1 = [W_si;0],
    # 2 = [W_sc;W_is], 3 = [W_in;0] -- pairs contract as one K=128 matmul
    wg = np.zeros((128, 4 * cfg.H), np.float16)
    WI = {r: np.asarray(inputs[f'W_{r}'], np.float32) for r in RELS}
    wg[0:64, 0:128] = WI['ni']
    wg[64:128, 0:128] = WI['ii']
    wg[0:64, 128:256] = WI['si']
    wg[0:64, 256:384] = WI['sc']
    wg[64:128, 256:384] = WI['is']
    wg[0:64, 384:512] = WI['in']
    bg = np.zeros((P, 3), np.float32)
    for ty, rels in TYPE_RELS.items():
        bsum = sum(np.asarray(inputs[f'b_{r}'], np.float32) for r in rels)
        bg[:, TYPE_IDX[ty]] = bsum / len(rels)

    # LSTM gate rows reordered i,f,g,o -> f,i,o,g.  sigmoid(o) is computed as
    # tanh(o/2) alongside tanh(g) in one scale-vectored ACT op; the missing
    # (x+1)/2 affine is recovered by computing h' = 2h and folding 0.5 into
    # every consumer of h (Whh0, Wih1, Whh1 and the output ReLU scale).
    perm = np.r_[64:128, 0:64, 192:256, 128:192]
    wx0 = np.ascontiguousarray(
        np.asarray(inputs['Wih0'], np.float32).T[:, perm]).astype(np.float16)
    wr = np.ascontiguousarray(
        np.asarray(inputs['Whh0'], np.float32).T[:, perm] * 0.5
    ).astype(np.float16)  # [O, 4O]
    # layer1 x-input (h0) and recurrent (h1) weights stacked on K so both
    # contract in one K=128 matmul against the combined [h0;h1] state tile
    wl1 = np.ascontiguousarray(np.vstack(
        [np.asarray(inputs['Wih1'], np.float32).T[:, perm] * 0.5,
         np.asarray(inputs['Whh1'], np.float32).T[:, perm] * 0.5]
    )).astype(np.float16)  # [2O, 4O]
    b0 = (np.asarray(inputs['bih0'], np.float32)
          + np.asarray(inputs['bhh0'], np.float32))[perm]
    b1 = (np.asarray(inputs['bih1'], np.float32)
          + np.asarray(inputs['bhh1'], np.float32))[perm]
    b0[128:192] *= 0.5  # o-gate bias halved (tanh(x/2) trick)
    b1[128:192] *= 0.5
    tsc = np.concatenate([np.full(64, 0.5), np.ones(64)]).astype(np.float32)
    bl = np.stack([b0[:P], b0[P:], b1[:P], b1[P:], tsc], axis=1).astype(np.float32)

    xt = {ty: np.ascontiguousarray(
        np.asarray(inputs[f'x_{ty}'], np.float32)
        .reshape(cfg.n[ty], cfg.TF).astype(np.float16))
        for ty in ('node', 'pod', 'svc')}

    in_maps = []
    for c in range(cfg.ncores):
        in_maps.append({
            'x_node': xt['node'], 'x_pod': xt['pod'], 'x_svc': xt['svc'],
            'gidx': np.ascontiguousarray(gidx[c].T),
            's_all': s_all[c],
            'w_gc': wg, 'b_gc': bg, 'wx0': wx0, 'wr': wr, 'wl1': wl1,
            'b_lstm': bl,
        })

    # local row -> global output row map
    rowmap = np.full((cfg.ncores, cfg.nloc), -1, np.int64)
    gbase = {'node': 0, 'pod': cfg.n['node'], 'svc': cfg.n['node'] + cfg.n['pod']}
    for ty in ('node', 'pod', 'svc'):
        a, s = assign[ty], slot[ty]
        core = a // cfg.nb[ty]
        loc = (cfg.blk0[ty] + a % cfg.nb[ty]) * P + s
        rowmap[core, loc] = gbase[ty] + np.arange(cfg.n[ty])
    return K, qtot, in_maps, rowmap


def build_program(cfg, K, qtot):
    T, TF, TH, TO = cfg.T, cfg.TF, cfg.TH, cfg.TO
    nc = bacc.Bacc("TRN2", target_bir_lowering=False, debug=False,
                   num_devices=cfg.ncores)
    x = {ty: nc.dram_tensor(f"x_{ty}", [cfg.n[ty], TF], f16, kind="ExternalInput")
         for ty in ('node', 'pod', 'svc')}
    gidx = nc.dram_tensor("gidx", [P, qtot], i32, kind="ExternalInput")
    s_all = nc.dram_tensor("s_all", [qtot, P, P], f16, kind="ExternalInput")
    w_gc = nc.dram_tensor("w_gc", [P, 4 * cfg.H], f16, kind="ExternalInput")
    b_gc = nc.dram_tensor("b_gc", [P, 3], f32, kind="ExternalInput")
    wx0 = nc.dram_tensor("wx0", [P, 256], f16, kind="ExternalInput")
    wr = nc.dram_tensor("wr", [64, 256], f16, kind="ExternalInput")
    wl1 = nc.dram_tensor("wl1", [P, 256], f16, kind="ExternalInput")
    b_lstm = nc.dram_tensor("b_lstm", [P, 5], f32, kind="ExternalInput")
    out_feat = nc.dram_tensor("out_feat", [cfg.nloc, TH], f32, kind="ExternalOutput")
    out_h = nc.dram_tensor("out_h", [cfg.nloc, TO], f32, kind="ExternalOutput")

    NW = cfg.maxg * P  # LSTM free width (512)

    with tile.TileContext(nc) as tc:
        with (tc.tile_pool(name="const", bufs=1) as pc,
              tc.tile_pool(name="gio", bufs=4) as pg,
              tc.tile_pool(name="feat", bufs=3) as pw,
              tc.tile_pool(name="stage", bufs=2) as pw2,
              tc.tile_pool(name="mts", bufs=3) as pm,
              tc.tile_pool(name="state", bufs=2) as pst,
              tc.tile_pool(name="ew", bufs=4) as pl3,
              tc.tile_pool(name="htp", bufs=1, space="PSUM") as ppt,
              tc.tile_pool(name="gpsum", bufs=2, space="PSUM") as ppg,
              tc.tile_pool(name="mpsum", bufs=1, space="PSUM") as ppm,
              tc.tile_pool(name="pjp", bufs=1, space="PSUM") as ppj):
            idx_all = pc.tile([P, qtot], i32)
            nc.sync.dma_start(idx_all[:], gidx[:])
            wgc_t = pc.tile([P, 4 * cfg.H], f16)
            nc.sync.dma_start(wgc_t[:], w_gc[:])
            bgc_t = pc.tile([P, 3], f32)
            nc.sync.dma_start(bgc_t[:], b_gc[:])
            wx0_t = pc.tile([P, 256], f16)
            nc.sync.dma_start(wx0_t[:], wx0[:])
            wr_t = pc.tile([64, 256], f16)
            nc.sync.dma_start(wr_t[:], wr[:])
            wl1_t = pc.tile([P, 256], f16)
            nc.sync.dma_start(wl1_t[:], wl1[:])
            bl_t = pc.tile([P, 5], f32)
            nc.sync.dma_start(bl_t[:], b_lstm[:])
            ident = pc.tile([P, P], f16)
            make_identity(nc, ident[:])

        # ---- phase A+B for one block group: gather/aggregate/project ----
            qref = [0]

            def phase_ab(blks, featT):
                featT_v = featT[:].rearrange("p (t w) -> p t w", w=NW)
                for bi, b in enumerate(blks):
                    bt = cfg.block_type(b)
                    rels = TYPE_RELS[bt]
                    # mT targets: relation pairs stacked on partitions so the
                    # projection contracts both in one K=128 matmul
                    if bt == 'pod':
                        mtp = pm.tile([P, 2 * TF], f16, tag="mtp", name=f"mtp{b}")
                        mts1 = pm.tile([64, 2 * TF], f16, tag="mts", name=f"mts{b}")
                        targets = {'ni': (mtp, 0), 'ii': (mtp, 64), 'si': (mts1, 0)}
                        proj = [(mtp, 0, P), (mts1, P, 64)]
                    elif bt == 'svc':
                        mtp = pm.tile([P, 2 * TF], f16, tag="mtp", name=f"mtp{b}")
                        targets = {'sc': (mtp, 0), 'is': (mtp, 64)}
                        proj = [(mtp, 2 * P, P)]
                    else:
                        mts1 = pm.tile([64, 2 * TF], f16, tag="mts", name=f"mts{b}")
                        targets = {'in': (mts1, 0)}
                        proj = [(mts1, 3 * P, 64)]
                    for r in rels:
                        mpsum = ppm.tile([P, TF], f32, tag="mpsum")
                        Kbr = K[b][r]
                        for k in range(Kbr):
                            q = qref[0]
                            g = pg.tile([P, TF], f16, tag="g")
                            nc.gpsimd.indirect_dma_start(
                                out=g[:], out_offset=None,
                                in_=x[RELS[r][0]][:],
                                in_offset=bass.IndirectOffsetOnAxis(
                                    ap=idx_all[:, q:q + 1], axis=0))
                            st = pg.tile([P, P], f16, tag="s")
                            nc.sync.dma_start(st[:], s_all[q])
                            # psum "start" clears a whole 2KB bank: only
                            # bank-first slices start, bank-last slices stop
                            for j in range(TF // P):
                                nc.tensor.matmul(
                                    mpsum[:, j * P:(j + 1) * P],
                                    lhsT=g[:, j * P:(j + 1) * P], rhs=st[:],
                                    start=(k == 0 and j % 4 == 0),
                                    stop=(k == Kbr - 1 and j % 4 == 3))
                            qref[0] += 1
                        # de-interleave [t_even f | t_odd f] psum rows into the
                        # base-0 stacked tile (16-bit matmul operands at
                        # partition base 64 fault on HW)
                        tgt, row0 = targets[r]
                        tv = tgt[row0:row0 + 64, :].rearrange(
                            "p (t2 two d) -> p t2 two d", two=2, d=P)
                        mtmp = pg.tile([P, TF], f16, tag="mtmp")
                        nc.vector.tensor_copy(mtmp[:], mpsum[:])
                        nc.sync.dma_start(
                            tv[:, :, 0, :],
                            mtmp[0:64, :].rearrange("p (j d) -> p j d", d=P))
                        nc.sync.dma_start(
                            tv[:, :, 1, :],
                            mtmp[64:128, :].rearrange("p (j d) -> p j d", d=P))
                    stf = pw2.tile([P, TH], f32, tag="stf")
                    ty = TYPE_IDX[bt]
                    for tq in range(T // 4):
                        pj = ppj.tile([P, 512], f32, tag="pj")
                        # mt columns are t-major so 4 timesteps project in one
                        # N=512 matmul per weight piece
                        for pi2, (mtile, wc, kk) in enumerate(proj):
                            nc.tensor.matmul(
                                pj[:],
                                lhsT=wgc_t[0:kk, wc:wc + P],
                                rhs=mtile[0:kk, tq * 512:(tq + 1) * 512],
                                start=(pi2 == 0),
                                stop=(pi2 == len(proj) - 1))
                        nc.scalar.activation(
                            featT_v[:, tq * 4:(tq + 1) * 4, bi * P:(bi + 1) * P],
                            pj[:].rearrange("p (t w) -> p t w", w=P),
                            AF.Relu, bias=bgc_t[:, ty:ty + 1],
                            scale=1.0 / len(rels))
                        f2 = ppj.tile([P, 512], f16, tag="pj")
                        for tt in range(4):
                            t = tq * 4 + tt
                            nc.tensor.transpose(
                                out=f2[:, tt * P:(tt + 1) * P],
                                in_=featT_v[:, t, bi * P:(bi + 1) * P],
                                identity=ident[:])
                        nc.vector.tensor_copy(stf[:, tq * 512:(tq + 1) * 512],
                                              f2[:])
                    nc.sync.dma_start(out_feat[b * P:(b + 1) * P, :], stf[:])

            def lstm_pair_layer(pairst, t, layer, ga_pool, gb_pool):
                chains, hh, c0p, c1p, WT = pairst
                cur, nxt = hh[t % 2], hh[(t + 1) % 2]
                cp = c0p if layer == 0 else c1p
                bcol = 0 if layer == 0 else 2
                ga = ga_pool.tile([P, 2 * NW], f32, tag="gate",
                                  name=f"ga{t}_{layer}")
                gb = gb_pool.tile([P, 2 * NW], f32, tag="gate",
                                  name=f"gb{t}_{layer}")
                for ci, (featT, blks, sth) in enumerate(chains):
                    Wn = len(blks) * P
                    base = ci * NW
                    if layer == 0:
                        xin = featT[:, t * NW: t * NW + Wn]
                        hst = hh[(t + 1) % 2][0:64, base:base + Wn]
                        nc.tensor.matmul(ga[:, base:base + Wn],
                                         lhsT=wx0_t[:, 0:128], rhs=xin,
                                         start=True, stop=False)
                        nc.tensor.matmul(ga[:, base:base + Wn],
                                         lhsT=wr_t[:, 0:128], rhs=hst,
                                         start=False, stop=True)
                        nc.tensor.matmul(gb[:, base:base + Wn],
                                         lhsT=wx0_t[:, 128:256], rhs=xin,
                                         start=True, stop=False)
                        nc.tensor.matmul(gb[:, base:base + Wn],
                                         lhsT=wr_t[:, 128:256], rhs=hst,
                                         start=False, stop=True)
                    else:
                        # [h0(t); h1(t-1)] stacked: one K=128 matmul per tile
                        hst = cur[:, base:base + Wn]
                        nc.tensor.matmul(ga[:, base:base + Wn],
                                         lhsT=wl1_t[:, 0:128], rhs=hst,
                                         start=True, stop=True)
                        nc.tensor.matmul(gb[:, base:base + Wn],
                                         lhsT=wl1_t[:, 128:256], rhs=hst,
                                         start=True, stop=True)
                # paired elementwise: sif=[sig_f;sig_i], tgo=[tanh(o/2);tanh(g)]
                sifp = pl3.tile([P, 2 * NW], f16, tag="sif")
                nc.scalar.activation(sifp[:, :WT], ga[:, :WT], AF.Sigmoid,
                                     bias=bl_t[:, bcol:bcol + 1])
                tgop = pl3.tile([P, 2 * NW], f16, tag="tgo")
                nc.scalar.activation(tgop[:, :WT], gb[:, :WT], AF.Tanh,
                                     bias=bl_t[:, bcol + 1:bcol + 2],
                                     scale=bl_t[:, 4:5])
                prodG = pl3.tile([64, 2 * NW], f16, tag="prodG")
                nc.vector.tensor_mul(prodG[:, :WT], sifp[64:128, :WT],
                                     tgop[64:128, :WT])
                prodC = pl3.tile([64, 2 * NW], f16, tag="prodC")
                nc.vector.tensor_mul(prodC[:, :WT], sifp[0:64, :WT],
                                     cp[:, :WT])
                nc.vector.tensor_add(cp[:, :WT], prodG[:, :WT], prodC[:, :WT])
                tancp = pl3.tile([64, 2 * NW], f16, tag="tanc")
                nc.scalar.activation(tancp[:, :WT], cp[:, :WT], AF.Tanh)
                # h' = 2h = tanh(o/2)*tanh(c) + tanh(c); consumers carry the 0.5
                if layer == 0:
                    nc.vector.tensor_mul(cur[0:64, :WT], tgop[0:64, :WT],
                                         tancp[:, :WT])
                    nc.vector.tensor_add(cur[0:64, :WT], cur[0:64, :WT],
                                         tancp[:, :WT])
                    return None
                # layer1: build h in a base-0 temp (the stacked tile's h1 rows
                # sit at partition base 64, illegal as a 2-input DVE operand),
                # then copy into the next combined state tile
                htmp = pl3.tile([64, 2 * NW], f16, tag="prodG",
                                name=f"htmp{t}_{chains[0][1][0]}")
                nc.vector.tensor_mul(htmp[:, :WT], tgop[0:64, :WT],
                                     tancp[:, :WT])
                nc.vector.tensor_add(htmp[:, :WT], htmp[:, :WT],
                                     tancp[:, :WT])
                nc.vector.tensor_copy(nxt[64:128, :WT], htmp[:, :WT])
                return htmp

            def lstm_pair_hout(pairst, t, htmp):
                chains, hh, c0p, c1p, WT = pairst
                for ci, (featT, blks, sth) in enumerate(chains):
                    ng = len(blks)
                    ht = ppt.tile([P, ng * 64], f16, tag="ht")
                    for bi in range(ng):
                        c0 = ci * NW + bi * P
                        nc.tensor.transpose(
                            out=ht[:, bi * 64:(bi + 1) * 64],
                            in_=htmp[:, c0:c0 + P],
                            identity=ident[0:64, 0:64])
                    nc.scalar.activation(
                        sth[:].rearrange("p (m k) -> p m k", k=TO)[
                            :, 0:ng, t * 64:(t + 1) * 64],
                        ht[:].rearrange("p (m k) -> p m k", k=64),
                        AF.Relu, scale=0.5)

            # process chunks in pairs: build featT for both, then run the two
            # LSTM chains in lockstep, layers software-pipelined so L0(t+1)
            # overlaps L1(t).  The last pair's LSTM runs after all gather/
            # projection work, so its extra gate slot comes from the freed
            # mpsum/pj banks (scoped pools).
            def prep_pair(pair):
                chains = []
                for blks in pair:
                    featT = pw.tile([P, T * NW], f16, tag="featT",
                                    name=f"featT{blks[0]}")
                    phase_ab(blks, featT)
                    sth = pw2.tile([P, cfg.maxg * TO], f32, tag="sth",
                                   name=f"sth{blks[0]}")
                    chains.append((featT, blks, sth))
                return chains

            def run_pair(pair, chains, ga1_pool, gb1_pool):
                pid = pair[0][0]
                hh = [pst.tile([P, 2 * NW], f16, tag=f"hh{i}",
                               name=f"hh{i}_{pid}") for i in range(2)]
                c0p = pst.tile([64, 2 * NW], f16, tag="c0", name=f"c0_{pid}")
                c1p = pst.tile([64, 2 * NW], f16, tag="c1", name=f"c1_{pid}")
                for z in (hh[0], hh[1], c0p, c1p):
                    nc.gpsimd.memset(z[:], 0.0)
                WT = (len(pair) - 1) * NW + len(pair[-1]) * P
                ps = (chains, hh, c0p, c1p, WT)
                lstm_pair_layer(ps, 0, 0, ppg, ppg)
                for t in range(1, T):
                    lstm_pair_layer(ps, t, 0, ppg, ppg)
                    htmp = lstm_pair_layer(ps, t - 1, 1, ga1_pool, gb1_pool)
                    lstm_pair_hout(ps, t - 1, htmp)
                htmp = lstm_pair_layer(ps, T - 1, 1, ga1_pool, gb1_pool)
                lstm_pair_hout(ps, T - 1, htmp)
                for featT, blks, sth in chains:
                    for bi, b in enumerate(blks):
                        nc.sync.dma_start(out_h[b * P:(b + 1) * P, :],
                                          sth[:, bi * TO:(bi + 1) * TO])

            pairs = [cfg.chunks[i:i + 2] for i in range(0, len(cfg.chunks), 2)]
            for pair in pairs:
                run_pair(pair, prep_pair(pair), ppg, ppg)
            assert qref[0] == qtot
    nc.compile()
    return nc


_CACHE = {}


def _get_program(cfg, K, qtot):
    key = (cfg.nblk, cfg.T, cfg.ncores, qtot,
           tuple(tuple(sorted(kb.items())) for kb in K))
    if key not in _CACHE:
        _CACHE[key] = build_program(cfg, K, qtot)
    return _CACHE[key]


TRACE = False
_LAST = {}


def kernel(**inputs):
    cfg = CFG()
    K, qtot, in_maps, rowmap = preprocess(cfg, inputs)
    nc = _get_program(cfg, K, qtot)
    kw = {}
    if TRACE:
        kw = dict(trace=True)
    res = run_bass_kernel_spmd(nc, in_maps, core_ids=list(range(cfg.ncores)), **kw)
    _LAST['res'] = res

    n_total = sum(cfg.n.values())
    feat = np.zeros((n_total, cfg.T, cfg.H), np.float32)
    h = np.zeros((n_total, cfg.T, cfg.O), np.float32)
    for c in range(cfg.ncores):
        valid = rowmap[c] >= 0
        rows = rowmap[c][valid]
        feat[rows] = res.results[c]['out_feat'][valid].reshape(-1, cfg.T, cfg.H)
        h[rows] = res.results[c]['out_h'][valid].reshape(-1, cfg.T, cfg.O)
    return feat, h
